# revision 11
# baseline (speedup 1.0000x reference)
"""BinaryLinear Trainium2 kernel: out = sign(x) @ sign(W).T

x: (4, 4096, 1024) f32, W: (1024, 1024) f32 -> out (4, 4096, 1024) f32.

Strategy (8 NeuronCores, data-parallel over flattened batch*seq):
  - Each core gets a [2048, 1024] row-shard of x and the full W.
  - Per core, per 128-row m-tile:
      DMA x tile [128m, 1024i] f32 -> ACT Sign (fp32 -> fp8e4, +-1/0 exact)
      -> xbar DMA transpose of the fp8 bytes viewed as u16 pairs, giving a
         [128p, 4c, 128m, 2b] layout where contraction index i = 256c + 2p + b
      -> 8 fp8 DoubleRow matmuls (K=256 each) accumulate [128m, 1024o] in PSUM
      -> DVE copy PSUM -> SBUF -> DMA out.
  - W is prepped once per core with the same sign+transpose transform plus a
    small on-chip reformat so o is contiguous (N=512 moving operand), keeping
    the exact same (p, c, b) -> i contraction mapping on both operands.

All arithmetic is exact: sign values are +-1/0 (exact in fp8e4) and the PE
accumulates in fp32, so results are exact integers <= 1024.
"""

import numpy as np

P = 128
K = 1024  # in_features
N = 1024  # out_features
N_CORES = 8
M_TOTAL = 4 * 4096
M_PER_CORE = M_TOTAL // N_CORES


def build_binary_linear(tc, out, x, w):
    """Emit the per-core Tile kernel.

    out: DRAM [M, 1024] f32, x: DRAM [M, 1024] f32, w: DRAM [1024, 1024] f32.
    """
    import concourse.mybir as mybir

    nc = tc.nc
    f32 = mybir.dt.float32
    fp8 = mybir.dt.float8e4
    u16 = mybir.dt.uint16
    Sign = mybir.ActivationFunctionType.Sign
    DR = mybir.MatmulPerfMode.DoubleRow

    M = x.shape[0]
    assert M % P == 0 and x.shape[1] == K and w.shape == (N, K)
    n_mtiles = M // P

    with (
        tc.tile_pool(name="wsb", bufs=1) as wpool,
        tc.tile_pool(name="wtmp", bufs=6) as wtmp,
        tc.tile_pool(name="xin", bufs=8) as xin_pool,
        tc.tile_pool(name="xt", bufs=8) as xt_pool,
        tc.tile_pool(name="osb", bufs=6) as out_pool,
        tc.tile_pool(name="ps", bufs=2, space="PSUM") as psum_pool,
    ):
        # ---- W prep (once): wT[p, cb*1024 + o] = sign(W)[o, i], i = 256c+2p+b,
        # cb = 2c + b ----
        wT = wpool.tile([P, 8 * N], fp8)
        for t in range(N // P):
            wf = wtmp.tile([P, K], f32, tag="wf32")
            nc.sync.dma_start(out=wf, in_=w[t * P : (t + 1) * P, :])
            w8 = wtmp.tile([P, K], fp8, tag="wfp8")
            nc.scalar.activation(out=w8, in_=wf, func=Sign)
            wt2 = wtmp.tile([P, K], fp8, tag="wt2")
            # [128o, 512 pairs] -> [128p, 4c, 128o]; out[p,c,o] = in[o, 128c+p]
            nc.scalar.dma_start_transpose(
                out=wt2.bitcast(u16).rearrange("p (c o) -> p c o", c=4),
                in_=w8.bitcast(u16),
            )
            # reformat (c, o, b) -> free offset (2c+b)*1024 + t*128 + o
            in_v = wt2.rearrange("p (c o b) -> p c o b", c=4, b=2)
            out_v = wT.rearrange("p (c b o) -> p c o b", c=4, b=2)[
                :, :, t * P : (t + 1) * P, :
            ]
            nc.vector.tensor_copy(out=out_v, in_=in_v)

        # view for matmul rhs slices: [p][jj][d][b][o]; cb = 4*jj + 2*d + b
        w5 = wT.rearrange("p (jj d b o) -> p jj d b o", jj=2, d=2, b=2)

        # ---- main loop over m-tiles ----
        for mt in range(n_mtiles):
            xf = xin_pool.tile([P, K], f32, tag="xf32")
            nc.sync.dma_start(out=xf, in_=x[mt * P : (mt + 1) * P, :])
            x8 = xin_pool.tile([P, K], fp8, tag="xfp8")
            nc.scalar.activation(out=x8, in_=xf, func=Sign)
            xt2 = xt_pool.tile([P, K], fp8, tag="xt2")
            nc.scalar.dma_start_transpose(
                out=xt2.bitcast(u16).rearrange("p (c m) -> p c m", c=4),
                in_=x8.bitcast(u16),
            )
            # xt2[p, c, m, b] = sign_x[m, 256c + 2p + b]
            x4 = xt2.rearrange("p (c m b) -> p c m b", c=4, b=2)

            osb = out_pool.tile([P, N], f32, tag="osb")
            ps = [
                psum_pool.tile([P, 512], f32, tag="ps0", name="ps0"),
                psum_pool.tile([P, 512], f32, tag="ps1", name="ps1"),
            ]
            for idx, (j, b) in enumerate(((0, 0), (0, 1), (1, 0), (1, 1))):
                lhsT = x4[:, 2 * j : 2 * j + 2, :, b]  # [p][c:2][m:128]
                for h in range(2):
                    nc.tensor.matmul(
                        ps[h],
                        lhsT=lhsT,
                        rhs=w5[:, j, :, b, h * 512 : (h + 1) * 512],
                        start=(idx == 0),
                        stop=(idx == 3),
                        perf_mode=DR,
                    )
            for h in range(2):
                nc.vector.tensor_copy(out=osb[:, h * 512 : (h + 1) * 512], in_=ps[h])
            nc.sync.dma_start(out=out[mt * P : (mt + 1) * P, :], in_=osb)


def _legalize_dma_waits(nc):
    """Walrus caps in-struct sem waits: DMA_DIRECT2D_XPOSE takes 1, DMACopy 2.

    Tile's sem assignment is not transitively minimal and can emit 2-4 waits
    on DMA instructions. Hoist the excess into InstEventSemaphore wait-only
    instructions inserted just before the DMA on its triggering queue. This
    is sound: the queue executes the hoisted wait strictly before pushing the
    DMA descriptor, so the dependency is enforced (more conservatively) at
    trigger time instead of ring-pop time.
    """
    import concourse.mybir as mybir

    limits = {
        "InstDmaTransposeAnt": 1,
        "InstDMACopy": 1,
        "InstTensorCopy": 1,
        "InstActivation": 1,
        "InstMatmult": 1,
        "InstLdweights": 1,
        "InstMemset": 1,
        "InstTensorTensor": 1,
        "InstDrain": 1,
    }
    n_hoisted = 0
    for f in nc.m.functions:
        for bb in f.blocks:
            new_list = []
            for ins in bb.instructions:
                lim = limits.get(type(ins).__name__)
                si = getattr(ins, "sync_info", None)
                waits = list(si.on_wait) if si is not None and si.on_wait else []
                if lim is not None and len(waits) > lim:
                    # keep data-producer (engine-sem) waits in-struct first,
                    # then the freshest DMA-lane waits; hoist the rest
                    def keep_rank(w):
                        is_lane = w.ant_name.startswith(
                            "DMAHW"
                        ) or w.ant_name.startswith("DMASW")
                        return (1 if is_lane else 0, -w.wait_value)

                    waits_sorted = sorted(waits, key=keep_rank)
                    keep, hoist = waits_sorted[:lim], waits_sorted[lim:]
                    for ci in range(0, len(hoist), 2):
                        chunk = hoist[ci : ci + 2]
                        ev = mybir.InstEventSemaphore(
                            name=f"{ins.name}-prewait{ci // 2}",
                            engine=ins.engine,
                            ins=[],
                            outs=[],
                            sync_info=mybir.SyncInfo(on_wait=chunk, on_update=[]),
                        )
                        nc.inst_map[ev.name] = ev
                        new_list.append(ev)
                        n_hoisted += len(chunk)
                    ins.sync_info = mybir.SyncInfo(
                        on_wait=keep, on_update=list(si.on_update or [])
                    )
                new_list.append(ins)
            bb.instructions[:] = new_list
    return n_hoisted


def _build_nc(m_per_core):
    import concourse.bass as bass
    import concourse.mybir as mybir
    from concourse import tile

    nc = bass.Bass("TRN2", target_bir_lowering=False)
    x_d = nc.dram_tensor("x", [m_per_core, K], mybir.dt.float32, kind="ExternalInput")
    w_d = nc.dram_tensor("W", [N, K], mybir.dt.float32, kind="ExternalInput")
    out_d = nc.dram_tensor(
        "out", [m_per_core, N], mybir.dt.float32, kind="ExternalOutput"
    )
    with tile.TileContext(nc) as tc:
        build_binary_linear(tc, out_d.ap(), x_d.ap(), w_d.ap())
    _legalize_dma_waits(nc)
    return nc


_cached = {}


def _get_nc(m_per_core):
    if m_per_core not in _cached:
        _cached[m_per_core] = _build_nc(m_per_core)
    return _cached[m_per_core]


def kernel(x, W, _trace=False):
    from concourse import bass_utils

    xf = np.ascontiguousarray(np.asarray(x, dtype=np.float32).reshape(M_TOTAL, K))
    wf = np.ascontiguousarray(np.asarray(W, dtype=np.float32))
    in_maps = [
        {"x": xf[i * M_PER_CORE : (i + 1) * M_PER_CORE], "W": wf}
        for i in range(N_CORES)
    ]
    nc = _get_nc(M_PER_CORE)
    res = bass_utils.run_bass_kernel_spmd(
        nc, in_maps, core_ids=list(range(N_CORES)), trace=_trace
    )
    out = np.concatenate([r["out"] for r in res.results], axis=0)
    out = out.reshape(4, 4096, N).astype(np.float32)
    if _trace:
        kernel.last_results = res
    return out
